# revision 16
# baseline (speedup 1.0000x reference)
"""Trainium2 Bass kernel for nn_DecoderRNN: serial LSTM over B*(T+1)=1024 steps
followed by a 32000-vocab softmax head.

Strategy (8 NeuronCores, SPMD single program):
 - The recurrence is inherently serial (state threads through all 1024 steps),
   so every core replicates it: per step, gates = W_hh @ h_{t-1} as 64 bf16
   [128x128]x[128x1] matmuls accumulated in PSUM (the x-projection is
   preloaded into PSUM with an identity matmul). Gate order is [g | i,f | o]
   so the tanh(g) / sigmoid(i,f) activations overlap the tail of the PE
   stream and sigmoid(o) lands last. Cell update is a fused
   [sig_i|sig_f] * [tanh_g|c] multiply + halves-add, then tanh(c) and
   h = sig_o * tanh(c). All gate PSUM tiles are double-buffered so each
   step's matmul burst is contiguous.
 - The per-step matmul burst is weight-load bound (FWL streams W_hh through
   the PE array every step). The PE clock is HAM-throttled to 1.2 GHz unless
   the array stays busy, so ND dummy weight-load/matmul pairs pad the
   PE-idle window while the serial ACT/DVE nonlinearity chain runs,
   keeping the clock gate at 2.4 GHz.
 - x-projection for all steps is one bf16 GEMM done on-device up front.
 - The softmax head is sharded BY STEPS: core c computes full-vocab logits,
   exp and normalization for steps [128c, 128c+128) only (selected via the
   partition-id register with one dynamic-offset copy), writing a
   [128, 32000] fp32 output block. No cross-core communication is needed:
   each core owns complete softmax rows. Host concatenates the 8 blocks.
 - Precision: bf16 for all GEMM inputs and exp storage; fp32 PSUM
   accumulation and cell state throughout.
"""
import sys

if "/opt/trn_rl_repo" not in sys.path:
    sys.path.insert(0, "/opt/trn_rl_repo")

import ml_dtypes
import numpy as np

import concourse.bass as bass
import concourse.tile as tile
from concourse import bacc, mybir

E, H, V = 256, 512, 32000
B, T = 16, 63
S = B * (T + 1)            # 1024 total steps
N_CORES = 8
NW = 500                   # vocab block width
NB = V // NW               # 64 vocab blocks
F32 = mybir.dt.float32
BF16 = mybir.dt.bfloat16
AF = mybir.ActivationFunctionType
ALU = mybir.AluOpType
BF = ml_dtypes.bfloat16

ND = 0                     # dummy filler pairs per step (measured: no help —
FD = 64                    # the weight-load path is NX-clock bound, not HAM)

# gate column groups after the host permutation [g, i, f, o]
# psG = cols 0:4 (g) ; psIF = cols 4:12 (i, f) ; psO = cols 12:16 (o)


def build_nc(steps=S):
    """Build the SPMD Bass program (identical on all cores; the partition-id
    register selects each core's step block in the softmax head)."""
    assert steps % N_CORES == 0
    sblk = steps // N_CORES
    nc = bacc.Bacc("TRN2", target_bir_lowering=False, debug=False,
                   num_devices=N_CORES)

    xsT_d = nc.dram_tensor("xsT", [128, 2, steps], BF16, kind="ExternalInput")
    wihT_d = nc.dram_tensor("wihT", [128, 32, 128], BF16, kind="ExternalInput")
    biasg_d = nc.dram_tensor("biasg", [128, 16], F32, kind="ExternalInput")
    whhT_d = nc.dram_tensor("whhT", [128, 64, 128], BF16, kind="ExternalInput")
    woutT_d = nc.dram_tensor("woutT", [4, 128, V], BF16, kind="ExternalInput")
    bout_d = nc.dram_tensor("bout", [1, V], BF16, kind="ExternalInput")
    ones_d = nc.dram_tensor("ones1", [1, 128], BF16, kind="ExternalInput")
    idn_d = nc.dram_tensor("idn", [128, 128], BF16, kind="ExternalInput")
    probs_d = nc.dram_tensor("probs", [sblk, V], F32, kind="ExternalOutput")

    from contextlib import ExitStack
    with tile.TileContext(nc) as tc, ExitStack() as ctx:
        with tc.tile_pool(name="const", bufs=1) as cpool:
            # ---- persistent SBUF ----
            xsT = cpool.tile([128, 2, steps], BF16)
            wihT = cpool.tile([128, 32, 128], BF16)
            biasg = cpool.tile([128, 16], F32)
            whhT = cpool.tile([128, 64, 128], BF16)
            xprojT = cpool.tile([128, 16, steps], BF16)
            hhist = cpool.tile([128, 4, steps], BF16)
            tgc = cpool.tile([128, 8], F32)       # [tanh(g) | c]
            gact = cpool.tile([128, 12], F32)     # [sig i | sig f | sig o]
            sc = cpool.tile([128, 4], F32)        # sig(2c)
            prod = cpool.tile([128, 8], F32)
            hblk = cpool.tile([128, 4, sblk], BF16)
            ones1 = cpool.tile([1, 128], BF16)
            idn = cpool.tile([128, 128], BF16)
            exps = cpool.tile([128, NB, NW], BF16)
            sums = cpool.tile([128, NB], F32)
            tot = cpool.tile([128, 1], F32)
            inv = cpool.tile([128, 1], F32)

            nc.sync.dma_start(xsT[:], xsT_d.ap())
            nc.sync.dma_start(wihT[:], wihT_d.ap())
            nc.sync.dma_start(biasg[:], biasg_d.ap())
            nc.sync.dma_start(whhT[:], whhT_d.ap())
            nc.sync.dma_start(ones1[:], ones_d.ap())
            nc.sync.dma_start(idn[:], idn_d.ap())
            nc.vector.memset(tgc[:, 4:8], 0.0)    # c0 = 0

            # ---- phase 1: x-projection GEMM (bf16 in, fp32 accum) ----
            nxp = (steps + 511) // 512
            with tc.tile_pool(name="xp_ps", bufs=2, space="PSUM") as xp_ps:
                for j in range(16):
                    for n2 in range(nxp):
                        w = min(512, steps - 512 * n2)
                        ps = xp_ps.tile([128, 512], F32)
                        for e in range(2):
                            nc.tensor.matmul(
                                ps[:, :w],
                                wihT[:, e * 16 + j, :],
                                xsT[:, e, 512 * n2:512 * n2 + w],
                                start=(e == 0), stop=(e == 1))
                        nc.scalar.activation(
                            xprojT[:, j, 512 * n2:512 * n2 + w], ps[:, :w],
                            AF.Identity, bias=biasg[:, j:j + 1])

            # ---- phase 3 W_out prefetch: DMA engines are idle during the
            # recurrence, so start streaming the first head blocks now ----
            NPF = 6
            woutT_r = woutT_d.ap().rearrange("k p v -> p k v")
            wpool = ctx.enter_context(tc.tile_pool(name="wout", bufs=NPF))
            wts = {}
            for n in range(NPF):
                wt = wpool.tile([128, 4, NW], BF16, name=f"wt_pf{n}",
                                tag="wt")
                eng = nc.sync if n % 2 == 0 else nc.gpsimd
                eng.dma_start(wt[:], woutT_r[:, :, n * NW:(n + 1) * NW])
                wts[n] = wt

            # ---- phase 2: serial LSTM recurrence ----
            groups = [(0, 4), (4, 16)]
            with tc.tile_pool(name="g_ps", bufs=2, space="PSUM") as g_ps:
                for t in range(steps):
                    if t == 0:
                        nc.scalar.activation(tgc[:, 0:4], xprojT[:, 0:4, 0],
                                             AF.Tanh)
                        nc.scalar.activation(gact[:, 0:12],
                                             xprojT[:, 4:16, 0], AF.Sigmoid)
                    else:
                        tiles = [g_ps.tile([128, hi - lo], F32,
                                           tag=f"ps{gi}", name=f"ps{gi}_{t}")
                                 for gi, (lo, hi) in enumerate(groups)]
                        # g-group first (its tanh fires mid-burst), then
                        # [i|f|o] whose sigmoid fires at burst end
                        for ps, (lo, hi) in zip(tiles, groups):
                            nc.tensor.matmul(ps[:], idn[:],
                                             xprojT[:, lo:hi, t],
                                             start=True, stop=False)
                            for j in range(lo, hi):
                                for k in range(4):
                                    nc.tensor.matmul(
                                        ps[:, j - lo:j - lo + 1],
                                        whhT[:, k * 16 + j, :],
                                        hhist[:, k, t - 1:t],
                                        start=False,
                                        stop=(j == hi - 1 and k == 3))
                        psG, psIFO = tiles
                        nc.scalar.activation(tgc[:, 0:4], psG[:], AF.Tanh)
                        nc.scalar.activation(gact[:, 0:12], psIFO[:],
                                             AF.Sigmoid)
                    # cell update: c = sig_f*c + sig_i*tanh_g, then
                    # h = sig_o * tanh(c)
                    nc.vector.tensor_mul(prod[:], gact[:, 0:8], tgc[:, 0:8])
                    nc.vector.tensor_add(tgc[:, 4:8], prod[:, 0:4],
                                         prod[:, 4:8])
                    nc.scalar.activation(sc[:], tgc[:, 4:8], AF.Tanh)
                    nc.vector.tensor_mul(hhist[:, :, t], sc[:],
                                         gact[:, 8:12])

            # ---- phase 3: per-core step-block softmax head ----
            cid = nc.vector.partition_id()
            off = cid * sblk
            nc.vector.tensor_copy(hblk[:], hhist[:, :, bass.ds(off, sblk)])
            with tc.tile_pool(name="lg_ps", bufs=2, space="PSUM") as lg_ps, \
                 tc.tile_pool(name="bout", bufs=3) as bpool, \
                 tc.tile_pool(name="outstage", bufs=3) as opool:
                for n in range(NB):
                    if n in wts:
                        wt = wts.pop(n)
                    else:
                        wt = wpool.tile([128, 4, NW], BF16,
                                        name=f"wt_{n}", tag="wt")
                        eng = nc.sync if n % 2 == 0 else nc.gpsimd
                        eng.dma_start(wt[:],
                                      woutT_r[:, :, n * NW:(n + 1) * NW])
                    bt = bpool.tile([1, NW], BF16)
                    nc.gpsimd.dma_start(bt[:],
                                        bout_d[0:1, n * NW:(n + 1) * NW])
                    ps = lg_ps.tile([128, NW], F32)
                    nc.tensor.matmul(ps[:sblk, :], ones1[0:1, 0:sblk], bt[:],
                                     start=True, stop=False)
                    for k in range(4):
                        nc.tensor.matmul(ps[:sblk, :], hblk[:, k, :],
                                         wt[:, k, :],
                                         start=False, stop=(k == 3))
                    nc.scalar.activation(exps[:sblk, n, :], ps[:sblk, :],
                                         AF.Exp,
                                         accum_out=sums[:sblk, n:n + 1])
                nc.vector.reduce_sum(tot[:sblk, :], sums[:sblk, :],
                                     axis=mybir.AxisListType.X)
                nc.vector.reciprocal(inv[:sblk, :], tot[:sblk, :])
                for n in range(NB):
                    ot = opool.tile([128, NW], F32)
                    nc.vector.tensor_scalar_mul(ot[:sblk, :],
                                                exps[:sblk, n, :],
                                                inv[:sblk, :])
                    eng = nc.sync if n % 2 == 0 else nc.gpsimd
                    eng.dma_start(probs_d.ap()[:, n * NW:(n + 1) * NW],
                                  ot[:sblk, :])
    nc.compile()
    return nc


def prep_inputs(features, captions, emb, W_ih, W_hh, b_ih, b_hh, W_out, b_out,
                steps=S):
    """Host-side packing: gather + transpose + gate permutation. Pure data
    movement; all FLOPs stay on device."""
    features = np.asarray(features, np.float32)
    captions = np.asarray(captions)
    emb = np.asarray(emb, np.float32)
    W_ih = np.asarray(W_ih, np.float32)
    W_hh = np.asarray(W_hh, np.float32)
    W_out = np.asarray(W_out, np.float32)
    b = np.asarray(b_ih, np.float32) + np.asarray(b_hh, np.float32)
    b_out = np.asarray(b_out, np.float32)

    # gate order [i,f,g,o] -> [g,i,f,o]
    perm = np.concatenate([np.arange(1024, 1536), np.arange(0, 512),
                           np.arange(512, 1024), np.arange(1536, 2048)])
    Wih_p = W_ih[perm]
    Whh_p = W_hh[perm]
    b_p = b[perm]

    xs = np.concatenate([features[:, None, :], emb[captions]], axis=1)
    xs = xs.reshape(S, E)[:steps]
    xsT = np.ascontiguousarray(
        xs.T.reshape(2, 128, steps).transpose(1, 0, 2)).astype(BF)  # [p,e,t]
    wihT = np.ascontiguousarray(
        Wih_p.T.reshape(2, 128, 16, 128).transpose(1, 0, 2, 3)
        .reshape(128, 32, 128)).astype(BF)                        # [p,(e,j),m]
    biasg = np.ascontiguousarray(b_p.reshape(16, 128).T)          # [p,j]
    whhT = np.ascontiguousarray(
        Whh_p.T.reshape(4, 128, 16, 128).transpose(1, 0, 2, 3)
        .reshape(128, 64, 128)).astype(BF)                        # [p,(k,j),m]
    woutT = np.ascontiguousarray(W_out.T.reshape(4, 128, V)).astype(BF)
    bout = b_out[None, :].astype(BF)
    ones1 = np.ones((1, 128), BF)
    idn = np.eye(128, dtype=np.float32).astype(BF)
    return {"xsT": xsT, "wihT": wihT, "biasg": biasg, "whhT": whhT,
            "woutT": woutT, "bout": bout, "ones1": ones1, "idn": idn}


_NC_CACHE = {}


def _get_nc(steps=S):
    if steps not in _NC_CACHE:
        _NC_CACHE[steps] = build_nc(steps)
    return _NC_CACHE[steps]


def kernel(**inputs):
    from concourse.bass_utils import run_bass_kernel_spmd
    nc = _get_nc(S)
    in_map = prep_inputs(**inputs)
    res = run_bass_kernel_spmd(nc, [dict(in_map) for _ in range(N_CORES)],
                               core_ids=list(range(N_CORES)))
    probs = np.concatenate([res.results[c]["probs"] for c in range(N_CORES)],
                           axis=0)
    return probs.reshape(B, T + 1, V).astype(np.float32)


# revision 21
# speedup vs baseline: 1.0566x; 1.0566x over previous
"""Trainium2 Bass kernel for nn_DecoderRNN: serial LSTM over B*(T+1)=1024 steps
followed by a 32000-vocab softmax head.

Strategy (8 NeuronCores, SPMD single program):
 - The recurrence is inherently serial (state threads through all 1024 steps),
   so every core replicates it: per step, gates = W_hh @ h_{t-1} as 64 bf16
   [128x128]x[128x1] matmuls accumulated in PSUM (the x-projection is
   preloaded into PSUM with an identity matmul). Gate order is [g | i,f | o]
   so the tanh(g) / sigmoid(i,f) activations overlap the tail of the PE
   stream and sigmoid(o) lands last. Cell update is a fused
   [sig_i|sig_f] * [tanh_g|c] multiply + halves-add, then tanh(c) and
   h = sig_o * tanh(c). All gate PSUM tiles are double-buffered so each
   step's matmul burst is contiguous.
 - The per-step matmul burst is weight-load bound (FWL streams W_hh through
   the PE array every step). The PE clock is HAM-throttled to 1.2 GHz unless
   the array stays busy, so ND dummy weight-load/matmul pairs pad the
   PE-idle window while the serial ACT/DVE nonlinearity chain runs,
   keeping the clock gate at 2.4 GHz.
 - x-projection for all steps is one bf16 GEMM done on-device up front.
 - The softmax head is sharded BY STEPS: core c computes full-vocab logits,
   exp and normalization for steps [128c, 128c+128) only (selected via the
   partition-id register with one dynamic-offset copy), writing a
   [128, 32000] fp32 output block. No cross-core communication is needed:
   each core owns complete softmax rows. Host concatenates the 8 blocks.
 - Precision: bf16 for all GEMM inputs and exp storage; fp32 PSUM
   accumulation and cell state throughout.
"""
import sys

if "/opt/trn_rl_repo" not in sys.path:
    sys.path.insert(0, "/opt/trn_rl_repo")

import ml_dtypes
import numpy as np

import concourse.bass as bass
import concourse.tile as tile
from concourse import bacc, mybir

E, H, V = 256, 512, 32000
B, T = 16, 63
S = B * (T + 1)            # 1024 total steps
N_CORES = 8
NW = 500                   # vocab block width
NB = V // NW               # 64 vocab blocks
F32 = mybir.dt.float32
BF16 = mybir.dt.bfloat16
AF = mybir.ActivationFunctionType
ALU = mybir.AluOpType
BF = ml_dtypes.bfloat16

ND = 6                     # trailing dummy pairs per step: absorb the
FD = 64                    # o-group's PSUM pipeline-drain so sig(o) fires fast

# gate column groups after the host permutation [g, i, f, o]
# psG = cols 0:4 (g) ; psIF = cols 4:12 (i, f) ; psO = cols 12:16 (o)


def build_nc(steps=S):
    """Build the SPMD Bass program (identical on all cores; the partition-id
    register selects each core's step block in the softmax head)."""
    assert steps % N_CORES == 0
    sblk = steps // N_CORES
    nc = bacc.Bacc("TRN2", target_bir_lowering=False, debug=False,
                   num_devices=N_CORES)

    xsT_d = nc.dram_tensor("xsT", [128, 2, steps], BF16, kind="ExternalInput")
    wihT_d = nc.dram_tensor("wihT", [128, 32, 128], BF16, kind="ExternalInput")
    biasg_d = nc.dram_tensor("biasg", [128, 16], F32, kind="ExternalInput")
    whhT_d = nc.dram_tensor("whhT", [128, 64, 128], BF16, kind="ExternalInput")
    woutT_d = nc.dram_tensor("woutT", [4, 128, V], BF16, kind="ExternalInput")
    bout_d = nc.dram_tensor("bout", [1, V], BF16, kind="ExternalInput")
    ones_d = nc.dram_tensor("ones1", [1, 128], BF16, kind="ExternalInput")
    idn_d = nc.dram_tensor("idn", [128, 128], BF16, kind="ExternalInput")
    probs_d = nc.dram_tensor("probs", [sblk, V], F32, kind="ExternalOutput")

    with tile.TileContext(nc) as tc:
        with tc.tile_pool(name="const", bufs=1) as cpool:
            # ---- persistent SBUF ----
            xsT = cpool.tile([128, 2, steps], BF16)
            wihT = cpool.tile([128, 32, 128], BF16)
            biasg = cpool.tile([128, 16], F32)
            whhT = cpool.tile([128, 64, 128], BF16)
            xprojT = cpool.tile([128, 16, steps], BF16)
            hhist = cpool.tile([128, 4, steps], BF16)
            tgc = cpool.tile([128, 8], F32)       # [tanh(g) | c]
            gact = cpool.tile([128, 12], F32)     # [sig i | sig f | sig o]
            sc = cpool.tile([128, 4], F32)        # sig(2c)
            prod = cpool.tile([128, 8], F32)
            hblk = cpool.tile([128, 4, sblk], BF16)
            ones1 = cpool.tile([1, 128], BF16)
            idn = cpool.tile([128, 128], BF16)
            exps = cpool.tile([128, NB, NW], BF16)
            sums = cpool.tile([128, NB], F32)
            tot = cpool.tile([128, 1], F32)
            inv = cpool.tile([128, 1], F32)

            nc.sync.dma_start(xsT[:], xsT_d.ap())
            nc.sync.dma_start(wihT[:], wihT_d.ap())
            nc.sync.dma_start(biasg[:], biasg_d.ap())
            nc.sync.dma_start(whhT[:], whhT_d.ap())
            nc.sync.dma_start(ones1[:], ones_d.ap())
            nc.sync.dma_start(idn[:], idn_d.ap())
            nc.vector.memset(tgc[:, 4:8], 0.0)    # c0 = 0

            # ---- phase 1: x-projection GEMM (bf16 in, fp32 accum) ----
            nxp = (steps + 511) // 512
            with tc.tile_pool(name="xp_ps", bufs=2, space="PSUM") as xp_ps:
                for j in range(16):
                    for n2 in range(nxp):
                        w = min(512, steps - 512 * n2)
                        ps = xp_ps.tile([128, 512], F32)
                        for e in range(2):
                            nc.tensor.matmul(
                                ps[:, :w],
                                wihT[:, e * 16 + j, :],
                                xsT[:, e, 512 * n2:512 * n2 + w],
                                start=(e == 0), stop=(e == 1))
                        nc.scalar.activation(
                            xprojT[:, j, 512 * n2:512 * n2 + w], ps[:, :w],
                            AF.Identity, bias=biasg[:, j:j + 1])

            # ---- phase 3 W_out prefetch: DMA engines are idle during the
            # recurrence, so start streaming the first head blocks now ----
            NPF = 6
            woutT_r = woutT_d.ap().rearrange("k p v -> p k v")
            wpool_cm = tc.tile_pool(name="wout", bufs=NPF)
            wpool = wpool_cm.__enter__()
            wts = {}
            for n in range(NPF):
                wt = wpool.tile([128, 4, NW], BF16, name=f"wt_pf{n}",
                                tag="wt")
                eng = nc.sync if n % 2 == 0 else nc.gpsimd
                eng.dma_start(wt[:], woutT_r[:, :, n * NW:(n + 1) * NW])
                wts[n] = wt

            # ---- phase 2: serial LSTM recurrence ----
            # group order [g | i,f | o]: tanh(g) fires mid-burst, sigmoid(i,f)
            # fires ~at burst end (the o-group absorbs its PSUM drain), and
            # ND trailing dummy pairs absorb the o-group's own drain so
            # sigmoid(o) fires promptly too.
            groups = [(0, 4), (4, 12), (12, 16)]
            with tc.tile_pool(name="g_ps", bufs=2, space="PSUM") as g_ps, \
                 tc.tile_pool(name="scr_ps", bufs=1, space="PSUM") as scr_ps:
                for t in range(steps):
                    if t == 0:
                        nc.scalar.activation(tgc[:, 0:4], xprojT[:, 0:4, 0],
                                             AF.Tanh)
                        nc.scalar.activation(gact[:, 0:12],
                                             xprojT[:, 4:16, 0], AF.Sigmoid)
                    else:
                        tiles = [g_ps.tile([128, hi - lo], F32,
                                           tag=f"ps{gi}", name=f"ps{gi}_{t}")
                                 for gi, (lo, hi) in enumerate(groups)]
                        for ps, (lo, hi) in zip(tiles, groups):
                            nc.tensor.matmul(ps[:], idn[:],
                                             xprojT[:, lo:hi, t],
                                             start=True, stop=False)
                            for j in range(lo, hi):
                                for k in range(4):
                                    nc.tensor.matmul(
                                        ps[:, j - lo:j - lo + 1],
                                        whhT[:, k * 16 + j, :],
                                        hhist[:, k, t - 1:t],
                                        start=False,
                                        stop=(j == hi - 1 and k == 3))
                        psG, psIF, psO = tiles
                        nc.scalar.activation(tgc[:, 0:4], psG[:], AF.Tanh)
                        nc.scalar.activation(gact[:, 0:8], psIF[:],
                                             AF.Sigmoid)
                        nc.scalar.activation(gact[:, 8:12], psO[:],
                                             AF.Sigmoid)
                        if ND:
                            dsc = scr_ps.tile([128, FD], F32, tag="dscr",
                                              name=f"dscr_{t}")
                            for dmy in range(ND):
                                nc.tensor.matmul(
                                    dsc[:], whhT[:, dmy % 64, :],
                                    whhT[:, (dmy + 1) % 64, 0:FD],
                                    start=True, stop=True)
                    # cell update: c = sig_f*c + sig_i*tanh_g, then
                    # h = sig_o * tanh(c)
                    nc.vector.tensor_mul(prod[:], gact[:, 0:8], tgc[:, 0:8])
                    nc.vector.tensor_add(tgc[:, 4:8], prod[:, 0:4],
                                         prod[:, 4:8])
                    nc.scalar.activation(sc[:], tgc[:, 4:8], AF.Tanh)
                    nc.vector.tensor_mul(hhist[:, :, t], sc[:],
                                         gact[:, 8:12])

            # ---- phase 3: per-core step-block softmax head ----
            cid = nc.vector.partition_id()
            off = cid * sblk
            nc.vector.tensor_copy(hblk[:], hhist[:, :, bass.ds(off, sblk)])
            with tc.tile_pool(name="lg_ps", bufs=2, space="PSUM") as lg_ps, \
                 tc.tile_pool(name="bout", bufs=3) as bpool, \
                 tc.tile_pool(name="outstage", bufs=3) as opool:
                for n in range(NB):
                    if n in wts:
                        wt = wts.pop(n)
                    else:
                        wt = wpool.tile([128, 4, NW], BF16,
                                        name=f"wt_{n}", tag="wt")
                        eng = nc.sync if n % 2 == 0 else nc.gpsimd
                        eng.dma_start(wt[:],
                                      woutT_r[:, :, n * NW:(n + 1) * NW])
                    bt = bpool.tile([1, NW], BF16)
                    nc.gpsimd.dma_start(bt[:],
                                        bout_d[0:1, n * NW:(n + 1) * NW])
                    ps = lg_ps.tile([128, NW], F32)
                    nc.tensor.matmul(ps[:sblk, :], ones1[0:1, 0:sblk], bt[:],
                                     start=True, stop=False)
                    for k in range(4):
                        nc.tensor.matmul(ps[:sblk, :], hblk[:, k, :],
                                         wt[:, k, :],
                                         start=False, stop=(k == 3))
                    nc.scalar.activation(exps[:sblk, n, :], ps[:sblk, :],
                                         AF.Exp,
                                         accum_out=sums[:sblk, n:n + 1])
                nc.vector.reduce_sum(tot[:sblk, :], sums[:sblk, :],
                                     axis=mybir.AxisListType.X)
                nc.vector.reciprocal(inv[:sblk, :], tot[:sblk, :])
                for n in range(NB):
                    ot = opool.tile([128, NW], F32)
                    nc.vector.tensor_scalar_mul(ot[:sblk, :],
                                                exps[:sblk, n, :],
                                                inv[:sblk, :])
                    eng = nc.sync if n % 2 == 0 else nc.gpsimd
                    eng.dma_start(probs_d.ap()[:, n * NW:(n + 1) * NW],
                                  ot[:sblk, :])
            wpool_cm.__exit__(None, None, None)
    nc.compile()
    return nc


def prep_inputs(features, captions, emb, W_ih, W_hh, b_ih, b_hh, W_out, b_out,
                steps=S):
    """Host-side packing: gather + transpose + gate permutation. Pure data
    movement; all FLOPs stay on device."""
    features = np.asarray(features, np.float32)
    captions = np.asarray(captions)
    emb = np.asarray(emb, np.float32)
    W_ih = np.asarray(W_ih, np.float32)
    W_hh = np.asarray(W_hh, np.float32)
    W_out = np.asarray(W_out, np.float32)
    b = np.asarray(b_ih, np.float32) + np.asarray(b_hh, np.float32)
    b_out = np.asarray(b_out, np.float32)

    # gate order [i,f,g,o] -> [g,i,f,o]
    perm = np.concatenate([np.arange(1024, 1536), np.arange(0, 512),
                           np.arange(512, 1024), np.arange(1536, 2048)])
    Wih_p = W_ih[perm]
    Whh_p = W_hh[perm]
    b_p = b[perm]

    xs = np.concatenate([features[:, None, :], emb[captions]], axis=1)
    xs = xs.reshape(S, E)[:steps]
    xsT = np.ascontiguousarray(
        xs.T.reshape(2, 128, steps).transpose(1, 0, 2)).astype(BF)  # [p,e,t]
    wihT = np.ascontiguousarray(
        Wih_p.T.reshape(2, 128, 16, 128).transpose(1, 0, 2, 3)
        .reshape(128, 32, 128)).astype(BF)                        # [p,(e,j),m]
    biasg = np.ascontiguousarray(b_p.reshape(16, 128).T)          # [p,j]
    whhT = np.ascontiguousarray(
        Whh_p.T.reshape(4, 128, 16, 128).transpose(1, 0, 2, 3)
        .reshape(128, 64, 128)).astype(BF)                        # [p,(k,j),m]
    woutT = np.ascontiguousarray(W_out.T.reshape(4, 128, V)).astype(BF)
    bout = b_out[None, :].astype(BF)
    ones1 = np.ones((1, 128), BF)
    idn = np.eye(128, dtype=np.float32).astype(BF)
    return {"xsT": xsT, "wihT": wihT, "biasg": biasg, "whhT": whhT,
            "woutT": woutT, "bout": bout, "ones1": ones1, "idn": idn}


_NC_CACHE = {}


def _get_nc(steps=S):
    if steps not in _NC_CACHE:
        _NC_CACHE[steps] = build_nc(steps)
    return _NC_CACHE[steps]


def kernel(**inputs):
    from concourse.bass_utils import run_bass_kernel_spmd
    nc = _get_nc(S)
    in_map = prep_inputs(**inputs)
    res = run_bass_kernel_spmd(nc, [dict(in_map) for _ in range(N_CORES)],
                               core_ids=list(range(N_CORES)))
    probs = np.concatenate([res.results[c]["probs"] for c in range(N_CORES)],
                           axis=0)
    return probs.reshape(B, T + 1, V).astype(np.float32)


# revision 29
# speedup vs baseline: 1.1771x; 1.1140x over previous
"""Trainium2 Bass kernel for nn_DecoderRNN: serial LSTM over B*(T+1)=1024 steps
followed by a 32000-vocab softmax head.

Strategy (8 NeuronCores, SPMD single program):
 - The recurrence is inherently serial (state threads through all 1024 steps),
   so every core replicates it: per step, gates = W_hh @ h_{t-1} as 64 bf16
   [128x128]x[128x1] matmuls accumulated in PSUM (the x-projection is
   preloaded into PSUM with an identity matmul), then sigmoid + cell update
   on ACT/DVE. Gates live in three PSUM tiles ((i,g) | f | o) so the
   activation work for early gate groups overlaps the tail of the PE stream.
   tanh(g) is computed as 2*sigmoid(2a)-1 with the 2x folded into the host-
   packed weights, so the gate nonlinearity is a single sigmoid pass plus a
   cheap DVE affine. h history accumulates in SBUF already transposed
   ([hidden-part, step-free]) for the output GEMM.
 - x-projection for all steps is one fp32 GEMM done on-device up front.
 - The softmax head is sharded BY STEPS: core c computes full-vocab logits,
   exp and normalization for steps [128c, 128c+128) only (selected via the
   partition-id register with one dynamic-offset copy), writing a
   [128, 32000] fp32 output block. No cross-core communication is needed:
   each core owns complete softmax rows. Host concatenates the 8 blocks.
 - Precision: bf16 for W_hh/h matmuls, x-projection storage, logits GEMM and
   exp storage; fp32 PSUM accumulation and cell state throughout
   (measured end-to-end rel-err vs fp32 reference: ~3.5e-3).
"""
import sys

if "/opt/trn_rl_repo" not in sys.path:
    sys.path.insert(0, "/opt/trn_rl_repo")

from contextlib import ExitStack

import ml_dtypes
import numpy as np

import concourse.bass as bass
import concourse.tile as tile
from concourse import bacc, mybir

E, H, V = 256, 512, 32000
B, T = 16, 63
S = B * (T + 1)            # 1024 total steps
N_CORES = 8
NW = 500                   # vocab block width
NB = V // NW               # 64 vocab blocks
F32 = mybir.dt.float32
BF16 = mybir.dt.bfloat16
AF = mybir.ActivationFunctionType
ALU = mybir.AluOpType
BF = ml_dtypes.bfloat16

# gate column groups after the host permutation [i, g, f, o]
# psA = cols 0:8 (i, g) ; psB1 = cols 8:12 (f) ; psB2 = cols 12:16 (o)


def build_nc(steps=S):
    """Build the SPMD Bass program (identical on all cores; the partition-id
    register selects each core's step block in the softmax head)."""
    assert steps % N_CORES == 0
    sblk = steps // N_CORES
    nc = bacc.Bacc("TRN2", target_bir_lowering=False, debug=False,
                   num_devices=N_CORES)

    xsT_d = nc.dram_tensor("xsT", [128, 2, steps], BF16, kind="ExternalInput")
    wihT_d = nc.dram_tensor("wihT", [128, 32, 128], BF16,
                            kind="ExternalInput")
    biasg_d = nc.dram_tensor("biasg", [128, 16], F32, kind="ExternalInput")
    whhT_d = nc.dram_tensor("whhT", [128, 64, 128], BF16, kind="ExternalInput")
    woutT_d = nc.dram_tensor("woutT", [4, 128, V], BF16, kind="ExternalInput")
    bout_d = nc.dram_tensor("bout", [1, V], BF16, kind="ExternalInput")
    ones_d = nc.dram_tensor("ones1", [1, 128], BF16, kind="ExternalInput")
    idn_d = nc.dram_tensor("idn", [128, 128], BF16, kind="ExternalInput")
    probs_d = nc.dram_tensor("probs", [sblk, V], F32, kind="ExternalOutput")

    with tile.TileContext(nc) as tc:
        with ExitStack() as ctx:
            cpool = ctx.enter_context(tc.tile_pool(name="const", bufs=1))
            xp_ps = ctx.enter_context(
                tc.tile_pool(name="xp_ps", bufs=2, space="PSUM"))
            g_ps = ctx.enter_context(
                tc.tile_pool(name="g_ps", bufs=1, space="PSUM"))
            lg_ps = ctx.enter_context(
                tc.tile_pool(name="lg_ps", bufs=2, space="PSUM"))
            spool = ctx.enter_context(tc.tile_pool(name="step", bufs=3))
            wpool = ctx.enter_context(tc.tile_pool(name="wout", bufs=6))
            bpool = ctx.enter_context(tc.tile_pool(name="bout", bufs=3))
            opool = ctx.enter_context(tc.tile_pool(name="outstage", bufs=3))

            # ---- persistent SBUF ----
            xsT = cpool.tile([128, 2, steps], BF16)
            wihT = cpool.tile([128, 32, 128], BF16)
            biasg = cpool.tile([128, 16], F32)
            whhT = cpool.tile([128, 64, 128], BF16)
            xprojT = cpool.tile([128, 16, steps], BF16)
            hhist = cpool.tile([128, 4, steps], BF16)
            c_sb = cpool.tile([128, 4], F32)
            gact = cpool.tile([128, 16], F32)
            hblk = cpool.tile([128, 4, sblk], BF16)
            ones1 = cpool.tile([1, 128], BF16)
            idn = cpool.tile([128, 128], BF16)
            exps = cpool.tile([128, NB, NW], BF16)
            sums = cpool.tile([128, NB], F32)
            tot = cpool.tile([128, 1], F32)
            inv = cpool.tile([128, 1], F32)

            nc.sync.dma_start(xsT[:], xsT_d.ap())
            nc.sync.dma_start(wihT[:], wihT_d.ap())
            nc.sync.dma_start(biasg[:], biasg_d.ap())
            nc.sync.dma_start(whhT[:], whhT_d.ap())
            nc.sync.dma_start(ones1[:], ones_d.ap())
            nc.sync.dma_start(idn[:], idn_d.ap())
            nc.vector.memset(c_sb[:], 0.0)

            # W_out prefetch: DMA engines are idle during the recurrence,
            # so stream the first head blocks early on two queues
            NPF = 6
            woutT_r = woutT_d.ap().rearrange("k p v -> p k v")
            wts = {}
            for n in range(NPF):
                wt = wpool.tile([128, 4, NW], BF16, name=f"wt_pf{n}",
                                tag="wt")
                eng = nc.sync if n % 2 == 0 else nc.gpsimd
                eng.dma_start(wt[:], woutT_r[:, :, n * NW:(n + 1) * NW])
                wts[n] = wt

            # ---- phase 1: x-projection GEMM (bf16 in, fp32 accum) ----
            nxp = (steps + 511) // 512
            for j in range(16):
                for n2 in range(nxp):
                    w = min(512, steps - 512 * n2)
                    ps = xp_ps.tile([128, 512], F32)
                    for e in range(2):
                        nc.tensor.matmul(
                            ps[:, :w],
                            wihT[:, e * 16 + j, :],
                            xsT[:, e, 512 * n2:512 * n2 + w],
                            start=(e == 0), stop=(e == 1))
                    nc.scalar.activation(
                        xprojT[:, j, 512 * n2:512 * n2 + w], ps[:, :w],
                        AF.Identity, bias=biasg[:, j:j + 1])

            # ---- phase 2: serial LSTM recurrence ----
            # per-step gate tiles: psA=(i,g) cols 0:8, psB1=f 8:12, psB2=o 12:16
            groups = [(0, 8), (8, 12), (12, 16)]
            for t in range(steps):
                if t == 0:
                    # h_{-1} = 0: gates are just the x-projection
                    nc.scalar.activation(gact[:, 0:8], xprojT[:, 0:8, 0],
                                         AF.Sigmoid)
                    nc.scalar.activation(gact[:, 8:12], xprojT[:, 8:12, 0],
                                         AF.Sigmoid)
                    nc.scalar.activation(gact[:, 12:16], xprojT[:, 12:16, 0],
                                         AF.Sigmoid)
                else:
                    tiles = [g_ps.tile([128, hi - lo], F32, tag=f"ps{gi}",
                                       name=f"ps{gi}_{t}",
                                       bufs=(2 if gi == 0 else 1))
                             for gi, (lo, hi) in enumerate(groups)]
                    # x-projection preload (PE, runs during previous tail)
                    for ps, (lo, hi) in zip(tiles, groups):
                        nc.tensor.matmul(ps[:], idn[:],
                                         xprojT[:, lo:hi, t],
                                         start=True, stop=False)
                    # W_hh @ h matmuls, group-major so (i,g) closes first
                    for ps, (lo, hi) in zip(tiles, groups):
                        for j in range(lo, hi):
                            for k in range(4):
                                nc.tensor.matmul(
                                    ps[:, j - lo:j - lo + 1],
                                    whhT[:, k * 16 + j, :],
                                    hhist[:, k, t - 1:t],
                                    start=False,
                                    stop=(j == hi - 1 and k == 3))
                    for ps, (lo, hi) in zip(tiles, groups):
                        nc.scalar.activation(gact[:, lo:hi], ps[:],
                                             AF.Sigmoid)
                # g' = 2*sigmoid(2a_g) - 1 = tanh(a_g)
                gp = spool.tile([128, 4], F32, tag="gp")
                nc.vector.tensor_scalar(gp[:], gact[:, 4:8], 2.0, -1.0,
                                        ALU.mult, ALU.add)
                ig = spool.tile([128, 4], F32, tag="ig")
                nc.vector.tensor_mul(ig[:], gact[:, 0:4], gp[:])
                fc = spool.tile([128, 4], F32, tag="fc")
                nc.vector.tensor_mul(fc[:], gact[:, 8:12], c_sb[:])
                nc.vector.tensor_add(c_sb[:], ig[:], fc[:])
                tc_t = spool.tile([128, 4], F32, tag="tc")
                nc.scalar.activation(tc_t[:], c_sb[:], AF.Tanh)
                nc.vector.tensor_mul(hhist[:, :, t], gact[:, 12:16], tc_t[:])

            # ---- phase 3: per-core step-block softmax head ----
            cid = nc.vector.partition_id()
            off = cid * sblk
            nc.vector.tensor_copy(hblk[:], hhist[:, :, bass.ds(off, sblk)])
            for n in range(NB):
                if n in wts:
                    wt = wts.pop(n)
                else:
                    wt = wpool.tile([128, 4, NW], BF16, name=f"wt_{n}",
                                    tag="wt")
                    eng = nc.sync if n % 2 == 0 else nc.gpsimd
                    eng.dma_start(wt[:], woutT_r[:, :, n * NW:(n + 1) * NW])
                bt = bpool.tile([1, NW], BF16)
                nc.gpsimd.dma_start(bt[:], bout_d[0:1, n * NW:(n + 1) * NW])
                ps = lg_ps.tile([128, NW], F32)
                nc.tensor.matmul(ps[:sblk, :], ones1[0:1, 0:sblk], bt[:],
                                 start=True, stop=False)
                for k in range(4):
                    nc.tensor.matmul(ps[:sblk, :], hblk[:, k, :], wt[:, k, :],
                                     start=False, stop=(k == 3))
                nc.scalar.activation(exps[:sblk, n, :], ps[:sblk, :], AF.Exp,
                                     accum_out=sums[:sblk, n:n + 1])
            nc.vector.reduce_sum(tot[:sblk, :], sums[:sblk, :],
                                 axis=mybir.AxisListType.X)
            nc.vector.reciprocal(inv[:sblk, :], tot[:sblk, :])
            for n in range(NB):
                ot = opool.tile([128, NW], F32)
                nc.vector.tensor_scalar_mul(ot[:sblk, :], exps[:sblk, n, :],
                                            inv[:sblk, :])
                eng = nc.sync if n % 2 == 0 else nc.gpsimd
                eng.dma_start(probs_d.ap()[:, n * NW:(n + 1) * NW],
                              ot[:sblk, :])
    nc.compile()
    return nc


def prep_inputs(features, captions, emb, W_ih, W_hh, b_ih, b_hh, W_out, b_out,
                steps=S):
    """Host-side packing: gather + transpose + gate permutation. Pure data
    movement (plus the 2x fold for the tanh-via-sigmoid identity); all FLOPs
    stay on device."""
    features = np.asarray(features, np.float32)
    captions = np.asarray(captions)
    emb = np.asarray(emb, np.float32)
    W_ih = np.asarray(W_ih, np.float32)
    W_hh = np.asarray(W_hh, np.float32)
    W_out = np.asarray(W_out, np.float32)
    b = np.asarray(b_ih, np.float32) + np.asarray(b_hh, np.float32)
    b_out = np.asarray(b_out, np.float32)

    # gate order [i,f,g,o] -> [i,g,f,o]; double the g rows so that
    # tanh(a_g) = 2*sigmoid(2*a_g) - 1 needs only a sigmoid on device
    perm = np.concatenate([np.arange(0, 512), np.arange(1024, 1536),
                           np.arange(512, 1024), np.arange(1536, 2048)])
    scale = np.ones((2048, 1), np.float32)
    scale[512:1024] = 2.0
    Wih_p = W_ih[perm] * scale
    Whh_p = W_hh[perm] * scale
    b_p = b[perm] * scale[:, 0]

    xs = np.concatenate([features[:, None, :], emb[captions]], axis=1)
    xs = xs.reshape(S, E)[:steps]
    xsT = np.ascontiguousarray(
        xs.T.reshape(2, 128, steps).transpose(1, 0, 2)).astype(BF)  # [p,e,t]
    wihT = np.ascontiguousarray(
        Wih_p.T.reshape(2, 128, 16, 128).transpose(1, 0, 2, 3)
        .reshape(128, 32, 128)).astype(BF)                        # [p,(e,j),m]
    biasg = np.ascontiguousarray(b_p.reshape(16, 128).T)          # [p,j]
    whhT = np.ascontiguousarray(
        Whh_p.T.reshape(4, 128, 16, 128).transpose(1, 0, 2, 3)
        .reshape(128, 64, 128)).astype(BF)                        # [p,(k,j),m]
    woutT = np.ascontiguousarray(W_out.T.reshape(4, 128, V)).astype(BF)
    bout = b_out[None, :].astype(BF)
    ones1 = np.ones((1, 128), BF)
    idn = np.eye(128, dtype=np.float32).astype(BF)
    return {"xsT": xsT, "wihT": wihT, "biasg": biasg, "whhT": whhT,
            "woutT": woutT, "bout": bout, "ones1": ones1, "idn": idn}


_NC_CACHE = {}


def _get_nc(steps=S):
    if steps not in _NC_CACHE:
        _NC_CACHE[steps] = build_nc(steps)
    return _NC_CACHE[steps]


def kernel(**inputs):
    from concourse.bass_utils import run_bass_kernel_spmd
    nc = _get_nc(S)
    in_map = prep_inputs(**inputs)
    res = run_bass_kernel_spmd(nc, [dict(in_map) for _ in range(N_CORES)],
                               core_ids=list(range(N_CORES)))
    probs = np.concatenate([res.results[c]["probs"] for c in range(N_CORES)],
                           axis=0)
    return probs.reshape(B, T + 1, V).astype(np.float32)



# revision 30
# speedup vs baseline: 1.1778x; 1.0006x over previous
"""Trainium2 Bass kernel for nn_DecoderRNN: serial LSTM over B*(T+1)=1024 steps
followed by a 32000-vocab softmax head.

Strategy (8 NeuronCores, SPMD single program):
 - The recurrence is inherently serial (state threads through all 1024 steps),
   so every core replicates it: per step, gates = W_hh @ h_{t-1} as 64 bf16
   [128x128]x[128x1] matmuls accumulated in PSUM (the x-projection is
   preloaded into PSUM with an identity matmul), then sigmoid + cell update
   on ACT/DVE. Gates live in three PSUM tiles ((i,g) | f | o) so the
   activation work for early gate groups overlaps the tail of the PE stream.
   tanh(g) is computed as 2*sigmoid(2a)-1 with the 2x folded into the host-
   packed weights, so the gate nonlinearity is a single sigmoid pass plus a
   cheap DVE affine. h history accumulates in SBUF already transposed
   ([hidden-part, step-free]) for the output GEMM.
 - x-projection for all steps is one fp32 GEMM done on-device up front.
 - The softmax head is sharded BY STEPS: core c computes full-vocab logits,
   exp and normalization for steps [128c, 128c+128) only (selected via the
   partition-id register with one dynamic-offset copy), writing a
   [128, 32000] fp32 output block. No cross-core communication is needed:
   each core owns complete softmax rows. Host concatenates the 8 blocks.
 - Precision: bf16 for W_hh/h matmuls, x-projection storage, logits GEMM and
   exp storage; fp32 PSUM accumulation and cell state throughout
   (measured end-to-end rel-err vs fp32 reference: ~3.5e-3).
"""
import sys

if "/opt/trn_rl_repo" not in sys.path:
    sys.path.insert(0, "/opt/trn_rl_repo")

from contextlib import ExitStack

import ml_dtypes
import numpy as np

import concourse.bass as bass
import concourse.tile as tile
from concourse import bacc, mybir

E, H, V = 256, 512, 32000
B, T = 16, 63
S = B * (T + 1)            # 1024 total steps
N_CORES = 8
NW = 500                   # vocab block width
NB = V // NW               # 64 vocab blocks
F32 = mybir.dt.float32
BF16 = mybir.dt.bfloat16
AF = mybir.ActivationFunctionType
ALU = mybir.AluOpType
BF = ml_dtypes.bfloat16

# gate column groups after the host permutation [i, g, f, o]
# psA = cols 0:8 (i, g) ; psB1 = cols 8:12 (f) ; psB2 = cols 12:16 (o)


def build_nc(steps=S):
    """Build the SPMD Bass program (identical on all cores; the partition-id
    register selects each core's step block in the softmax head)."""
    assert steps % N_CORES == 0
    sblk = steps // N_CORES
    nc = bacc.Bacc("TRN2", target_bir_lowering=False, debug=False,
                   num_devices=N_CORES)

    xsT_d = nc.dram_tensor("xsT", [128, 2, steps], BF16, kind="ExternalInput")
    wihT_d = nc.dram_tensor("wihT", [128, 32, 128], BF16,
                            kind="ExternalInput")
    biasg_d = nc.dram_tensor("biasg", [128, 16], F32, kind="ExternalInput")
    whhT_d = nc.dram_tensor("whhT", [128, 64, 128], BF16, kind="ExternalInput")
    woutT_d = nc.dram_tensor("woutT", [4, 128, V], BF16, kind="ExternalInput")
    bout_d = nc.dram_tensor("bout", [1, V], BF16, kind="ExternalInput")
    ones_d = nc.dram_tensor("ones1", [1, 128], BF16, kind="ExternalInput")
    idn_d = nc.dram_tensor("idn", [128, 128], BF16, kind="ExternalInput")
    probs_d = nc.dram_tensor("probs", [sblk, V], F32, kind="ExternalOutput")

    with tile.TileContext(nc) as tc:
        with ExitStack() as ctx:
            cpool = ctx.enter_context(tc.tile_pool(name="const", bufs=1))
            xp_ps = ctx.enter_context(
                tc.tile_pool(name="xp_ps", bufs=2, space="PSUM"))
            g_ps = ctx.enter_context(
                tc.tile_pool(name="g_ps", bufs=1, space="PSUM"))
            lg_ps = ctx.enter_context(
                tc.tile_pool(name="lg_ps", bufs=2, space="PSUM"))
            spool = ctx.enter_context(tc.tile_pool(name="step", bufs=3))
            wpool = ctx.enter_context(tc.tile_pool(name="wout", bufs=10))
            bpool = ctx.enter_context(tc.tile_pool(name="bout", bufs=3))
            opool = ctx.enter_context(tc.tile_pool(name="outstage", bufs=3))

            # ---- persistent SBUF ----
            xsT = cpool.tile([128, 2, steps], BF16)
            wihT = cpool.tile([128, 32, 128], BF16)
            biasg = cpool.tile([128, 16], F32)
            whhT = cpool.tile([128, 64, 128], BF16)
            xprojT = cpool.tile([128, 16, steps], BF16)
            hhist = cpool.tile([128, 4, steps], BF16)
            c_sb = cpool.tile([128, 4], F32)
            gact = cpool.tile([128, 16], F32)
            hblk = cpool.tile([128, 4, sblk], BF16)
            ones1 = cpool.tile([1, 128], BF16)
            idn = cpool.tile([128, 128], BF16)
            exps = cpool.tile([128, NB, NW], BF16)
            sums = cpool.tile([128, NB], F32)
            tot = cpool.tile([128, 1], F32)
            inv = cpool.tile([128, 1], F32)

            nc.sync.dma_start(xsT[:], xsT_d.ap())
            nc.sync.dma_start(wihT[:], wihT_d.ap())
            nc.sync.dma_start(biasg[:], biasg_d.ap())
            nc.sync.dma_start(whhT[:], whhT_d.ap())
            nc.sync.dma_start(ones1[:], ones_d.ap())
            nc.sync.dma_start(idn[:], idn_d.ap())
            nc.vector.memset(c_sb[:], 0.0)

            # W_out prefetch: DMA engines are idle during the recurrence,
            # so stream the first head blocks early on two queues
            NPF = 10
            woutT_r = woutT_d.ap().rearrange("k p v -> p k v")
            wts = {}
            for n in range(NPF):
                wt = wpool.tile([128, 4, NW], BF16, name=f"wt_pf{n}",
                                tag="wt")
                eng = nc.sync if n % 2 == 0 else nc.gpsimd
                eng.dma_start(wt[:], woutT_r[:, :, n * NW:(n + 1) * NW])
                wts[n] = wt

            # ---- phase 1: x-projection GEMM (bf16 in, fp32 accum) ----
            nxp = (steps + 511) // 512
            for j in range(16):
                for n2 in range(nxp):
                    w = min(512, steps - 512 * n2)
                    ps = xp_ps.tile([128, 512], F32)
                    for e in range(2):
                        nc.tensor.matmul(
                            ps[:, :w],
                            wihT[:, e * 16 + j, :],
                            xsT[:, e, 512 * n2:512 * n2 + w],
                            start=(e == 0), stop=(e == 1))
                    nc.scalar.activation(
                        xprojT[:, j, 512 * n2:512 * n2 + w], ps[:, :w],
                        AF.Identity, bias=biasg[:, j:j + 1])

            # ---- phase 2: serial LSTM recurrence ----
            # per-step gate tiles: psA=(i,g) cols 0:8, psB1=f 8:12, psB2=o 12:16
            groups = [(0, 8), (8, 12), (12, 16)]
            for t in range(steps):
                if t == 0:
                    # h_{-1} = 0: gates are just the x-projection
                    nc.scalar.activation(gact[:, 0:8], xprojT[:, 0:8, 0],
                                         AF.Sigmoid)
                    nc.scalar.activation(gact[:, 8:12], xprojT[:, 8:12, 0],
                                         AF.Sigmoid)
                    nc.scalar.activation(gact[:, 12:16], xprojT[:, 12:16, 0],
                                         AF.Sigmoid)
                else:
                    tiles = [g_ps.tile([128, hi - lo], F32, tag=f"ps{gi}",
                                       name=f"ps{gi}_{t}",
                                       bufs=(2 if gi == 0 else 1))
                             for gi, (lo, hi) in enumerate(groups)]
                    # x-projection preload (PE, runs during previous tail)
                    for ps, (lo, hi) in zip(tiles, groups):
                        nc.tensor.matmul(ps[:], idn[:],
                                         xprojT[:, lo:hi, t],
                                         start=True, stop=False)
                    # W_hh @ h matmuls, group-major so (i,g) closes first
                    for ps, (lo, hi) in zip(tiles, groups):
                        for j in range(lo, hi):
                            for k in range(4):
                                nc.tensor.matmul(
                                    ps[:, j - lo:j - lo + 1],
                                    whhT[:, k * 16 + j, :],
                                    hhist[:, k, t - 1:t],
                                    start=False,
                                    stop=(j == hi - 1 and k == 3))
                    for ps, (lo, hi) in zip(tiles, groups):
                        nc.scalar.activation(gact[:, lo:hi], ps[:],
                                             AF.Sigmoid)
                # g' = 2*sigmoid(2a_g) - 1 = tanh(a_g)
                gp = spool.tile([128, 4], F32, tag="gp")
                nc.vector.tensor_scalar(gp[:], gact[:, 4:8], 2.0, -1.0,
                                        ALU.mult, ALU.add)
                ig = spool.tile([128, 4], F32, tag="ig")
                nc.vector.tensor_mul(ig[:], gact[:, 0:4], gp[:])
                fc = spool.tile([128, 4], F32, tag="fc")
                nc.vector.tensor_mul(fc[:], gact[:, 8:12], c_sb[:])
                nc.vector.tensor_add(c_sb[:], ig[:], fc[:])
                tc_t = spool.tile([128, 4], F32, tag="tc")
                nc.scalar.activation(tc_t[:], c_sb[:], AF.Tanh)
                nc.vector.tensor_mul(hhist[:, :, t], gact[:, 12:16], tc_t[:])

            # ---- phase 3: per-core step-block softmax head ----
            cid = nc.vector.partition_id()
            off = cid * sblk
            nc.vector.tensor_copy(hblk[:], hhist[:, :, bass.ds(off, sblk)])
            for n in range(NB):
                if n in wts:
                    wt = wts.pop(n)
                else:
                    wt = wpool.tile([128, 4, NW], BF16, name=f"wt_{n}",
                                    tag="wt")
                    eng = nc.sync if n % 2 == 0 else nc.gpsimd
                    eng.dma_start(wt[:], woutT_r[:, :, n * NW:(n + 1) * NW])
                bt = bpool.tile([1, NW], BF16)
                nc.gpsimd.dma_start(bt[:], bout_d[0:1, n * NW:(n + 1) * NW])
                ps = lg_ps.tile([128, NW], F32)
                nc.tensor.matmul(ps[:sblk, :], ones1[0:1, 0:sblk], bt[:],
                                 start=True, stop=False)
                for k in range(4):
                    nc.tensor.matmul(ps[:sblk, :], hblk[:, k, :], wt[:, k, :],
                                     start=False, stop=(k == 3))
                nc.scalar.activation(exps[:sblk, n, :], ps[:sblk, :], AF.Exp,
                                     accum_out=sums[:sblk, n:n + 1])
            nc.vector.reduce_sum(tot[:sblk, :], sums[:sblk, :],
                                 axis=mybir.AxisListType.X)
            nc.vector.reciprocal(inv[:sblk, :], tot[:sblk, :])
            for n in range(NB):
                ot = opool.tile([128, NW], F32)
                nc.vector.tensor_scalar_mul(ot[:sblk, :], exps[:sblk, n, :],
                                            inv[:sblk, :])
                eng = nc.sync if n % 2 == 0 else nc.gpsimd
                eng.dma_start(probs_d.ap()[:, n * NW:(n + 1) * NW],
                              ot[:sblk, :])
    nc.compile()
    return nc


def prep_inputs(features, captions, emb, W_ih, W_hh, b_ih, b_hh, W_out, b_out,
                steps=S):
    """Host-side packing: gather + transpose + gate permutation. Pure data
    movement (plus the 2x fold for the tanh-via-sigmoid identity); all FLOPs
    stay on device."""
    features = np.asarray(features, np.float32)
    captions = np.asarray(captions)
    emb = np.asarray(emb, np.float32)
    W_ih = np.asarray(W_ih, np.float32)
    W_hh = np.asarray(W_hh, np.float32)
    W_out = np.asarray(W_out, np.float32)
    b = np.asarray(b_ih, np.float32) + np.asarray(b_hh, np.float32)
    b_out = np.asarray(b_out, np.float32)

    # gate order [i,f,g,o] -> [i,g,f,o]; double the g rows so that
    # tanh(a_g) = 2*sigmoid(2*a_g) - 1 needs only a sigmoid on device
    perm = np.concatenate([np.arange(0, 512), np.arange(1024, 1536),
                           np.arange(512, 1024), np.arange(1536, 2048)])
    scale = np.ones((2048, 1), np.float32)
    scale[512:1024] = 2.0
    Wih_p = W_ih[perm] * scale
    Whh_p = W_hh[perm] * scale
    b_p = b[perm] * scale[:, 0]

    xs = np.concatenate([features[:, None, :], emb[captions]], axis=1)
    xs = xs.reshape(S, E)[:steps]
    xsT = np.ascontiguousarray(
        xs.T.reshape(2, 128, steps).transpose(1, 0, 2)).astype(BF)  # [p,e,t]
    wihT = np.ascontiguousarray(
        Wih_p.T.reshape(2, 128, 16, 128).transpose(1, 0, 2, 3)
        .reshape(128, 32, 128)).astype(BF)                        # [p,(e,j),m]
    biasg = np.ascontiguousarray(b_p.reshape(16, 128).T)          # [p,j]
    whhT = np.ascontiguousarray(
        Whh_p.T.reshape(4, 128, 16, 128).transpose(1, 0, 2, 3)
        .reshape(128, 64, 128)).astype(BF)                        # [p,(k,j),m]
    woutT = np.ascontiguousarray(W_out.T.reshape(4, 128, V)).astype(BF)
    bout = b_out[None, :].astype(BF)
    ones1 = np.ones((1, 128), BF)
    idn = np.eye(128, dtype=np.float32).astype(BF)
    return {"xsT": xsT, "wihT": wihT, "biasg": biasg, "whhT": whhT,
            "woutT": woutT, "bout": bout, "ones1": ones1, "idn": idn}


_NC_CACHE = {}


def _get_nc(steps=S):
    if steps not in _NC_CACHE:
        _NC_CACHE[steps] = build_nc(steps)
    return _NC_CACHE[steps]


def kernel(**inputs):
    from concourse.bass_utils import run_bass_kernel_spmd
    nc = _get_nc(S)
    in_map = prep_inputs(**inputs)
    res = run_bass_kernel_spmd(nc, [dict(in_map) for _ in range(N_CORES)],
                               core_ids=list(range(N_CORES)))
    probs = np.concatenate([res.results[c]["probs"] for c in range(N_CORES)],
                           axis=0)
    return probs.reshape(B, T + 1, V).astype(np.float32)



# revision 36
# speedup vs baseline: 1.1892x; 1.0097x over previous
"""Trainium2 Bass kernel for nn_DecoderRNN: serial LSTM over B*(T+1)=1024 steps
followed by a 32000-vocab softmax head.

Strategy (8 NeuronCores, SPMD single program):
 - The recurrence is inherently serial (state threads through all 1024 steps),
   so every core replicates it: per step, gates = W_hh @ h_{t-1} as 64 bf16
   [128x128]x[128x1] matmuls accumulated in PSUM (the x-projection is
   preloaded into PSUM with an identity matmul), then sigmoid + cell update
   on ACT/DVE. Gates live in three PSUM tiles ((i,g) | f | o) so the
   activation work for early gate groups overlaps the tail of the PE stream.
   tanh(g) is computed as 2*sigmoid(2a)-1 with the 2x folded into the host-
   packed weights, so the gate nonlinearity is a single sigmoid pass plus a
   cheap DVE affine. h history accumulates in SBUF already transposed
   ([hidden-part, step-free]) for the output GEMM.
 - x-projection for all steps is one fp32 GEMM done on-device up front.
 - The softmax head is sharded BY STEPS: core c computes full-vocab logits,
   exp and normalization for steps [128c, 128c+128) only (selected via the
   partition-id register with one dynamic-offset copy), writing a
   [128, 32000] fp32 output block. No cross-core communication is needed:
   each core owns complete softmax rows. Host concatenates the 8 blocks.
 - Precision: bf16 for W_hh/h matmuls, x-projection storage, logits GEMM and
   exp storage; fp32 PSUM accumulation and cell state throughout
   (measured end-to-end rel-err vs fp32 reference: ~3.5e-3).
"""
import sys

if "/opt/trn_rl_repo" not in sys.path:
    sys.path.insert(0, "/opt/trn_rl_repo")

from contextlib import ExitStack

import ml_dtypes
import numpy as np

import concourse.bass as bass
import concourse.tile as tile
from concourse import bacc, mybir

E, H, V = 256, 512, 32000
B, T = 16, 63
S = B * (T + 1)            # 1024 total steps
N_CORES = 8
NW = 500                   # vocab block width
NB = V // NW               # 64 vocab blocks
F32 = mybir.dt.float32
BF16 = mybir.dt.bfloat16
AF = mybir.ActivationFunctionType
ALU = mybir.AluOpType
BF = ml_dtypes.bfloat16

# gate column groups after the host permutation [i, g, f, o]
# psA = cols 0:8 (i, g) ; psB1 = cols 8:12 (f) ; psB2 = cols 12:16 (o)


def build_nc(steps=S):
    """Build the SPMD Bass program (identical on all cores; the partition-id
    register selects each core's step block in the softmax head)."""
    assert steps % N_CORES == 0
    sblk = steps // N_CORES
    nc = bacc.Bacc("TRN2", target_bir_lowering=False, debug=False,
                   num_devices=N_CORES)

    xsT_d = nc.dram_tensor("xsT", [128, 2, steps], BF16, kind="ExternalInput")
    wihT_d = nc.dram_tensor("wihT", [128, 32, 128], BF16,
                            kind="ExternalInput")
    biasg_d = nc.dram_tensor("biasg", [128, 16], F32, kind="ExternalInput")
    whhT_d = nc.dram_tensor("whhT", [128, 64, 128], BF16, kind="ExternalInput")
    woutT_d = nc.dram_tensor("woutT", [4, 128, V], BF16, kind="ExternalInput")
    bout_d = nc.dram_tensor("bout", [1, V], BF16, kind="ExternalInput")
    ones_d = nc.dram_tensor("ones1", [1, 128], BF16, kind="ExternalInput")
    idn_d = nc.dram_tensor("idn", [128, 128], BF16, kind="ExternalInput")
    probs_d = nc.dram_tensor("probs", [sblk, V], F32, kind="ExternalOutput")

    with tile.TileContext(nc) as tc:
        with ExitStack() as ctx:
            cpool = ctx.enter_context(tc.tile_pool(name="const", bufs=1))
            xp_ps = ctx.enter_context(
                tc.tile_pool(name="xp_ps", bufs=2, space="PSUM"))
            g_ps = ctx.enter_context(
                tc.tile_pool(name="g_ps", bufs=1, space="PSUM"))
            lg_ps = ctx.enter_context(
                tc.tile_pool(name="lg_ps", bufs=2, space="PSUM"))
            spool = ctx.enter_context(tc.tile_pool(name="step", bufs=3))
            wpool = ctx.enter_context(tc.tile_pool(name="wout", bufs=10))
            bpool = ctx.enter_context(tc.tile_pool(name="bout", bufs=3))
            opool = ctx.enter_context(tc.tile_pool(name="outstage", bufs=3))

            # ---- persistent SBUF ----
            xsT = cpool.tile([128, 2, steps], BF16)
            wihT = cpool.tile([128, 32, 128], BF16)
            biasg = cpool.tile([128, 16], F32)
            whhT = cpool.tile([128, 64, 128], BF16)
            xprojT = cpool.tile([128, 16, steps], BF16)
            hhist = cpool.tile([128, 4, steps], BF16)
            c_sb = cpool.tile([128, 4], F32)
            gact = cpool.tile([128, 16], F32)
            hblk = cpool.tile([128, 4, sblk], BF16)
            ones1 = cpool.tile([1, 128], BF16)
            idn = cpool.tile([128, 128], BF16)
            exps = cpool.tile([128, NB * NW], BF16)
            sums = cpool.tile([128, NB], F32)
            tot = cpool.tile([128, 1], F32)
            inv = cpool.tile([128, 1], F32)

            nc.sync.dma_start(xsT[:], xsT_d.ap())
            nc.sync.dma_start(wihT[:], wihT_d.ap())
            nc.sync.dma_start(biasg[:], biasg_d.ap())
            nc.sync.dma_start(whhT[:], whhT_d.ap())
            nc.sync.dma_start(ones1[:], ones_d.ap())
            nc.sync.dma_start(idn[:], idn_d.ap())
            nc.vector.memset(c_sb[:], 0.0)

            # ---- phase 1: x-projection GEMM (bf16 in, fp32 accum) ----
            nxp = (steps + 511) // 512
            for j in range(16):
                for n2 in range(nxp):
                    w = min(512, steps - 512 * n2)
                    ps = xp_ps.tile([128, 512], F32)
                    for e in range(2):
                        nc.tensor.matmul(
                            ps[:, :w],
                            wihT[:, e * 16 + j, :],
                            xsT[:, e, 512 * n2:512 * n2 + w],
                            start=(e == 0), stop=(e == 1))
                    nc.scalar.activation(
                        xprojT[:, j, 512 * n2:512 * n2 + w], ps[:, :w],
                        AF.Identity, bias=biasg[:, j:j + 1])

            # W_out prefetch: DMA engines are idle during the recurrence,
            # so stream the first head blocks now on two queues (emitted
            # after phase 1 so they don't contend with the input loads)
            NPF = 10
            woutT_r = woutT_d.ap().rearrange("k p v -> p k v")
            wts = {}
            for n in range(NPF):
                wt = wpool.tile([128, 4, NW], BF16, name=f"wt_pf{n}",
                                tag="wt")
                eng = nc.sync if n % 2 == 0 else nc.gpsimd
                eng.dma_start(wt[:], woutT_r[:, :, n * NW:(n + 1) * NW])
                wts[n] = wt

            # ---- phase 2: serial LSTM recurrence ----
            # per-step gate tiles: psA=(i,g) cols 0:8, psB1=f 8:12, psB2=o 12:16
            groups = [(0, 8), (8, 12), (12, 16)]
            for t in range(steps):
                if t == 0:
                    # h_{-1} = 0: gates are just the x-projection
                    nc.scalar.activation(gact[:, 0:8], xprojT[:, 0:8, 0],
                                         AF.Sigmoid)
                    nc.scalar.activation(gact[:, 8:12], xprojT[:, 8:12, 0],
                                         AF.Sigmoid)
                    nc.scalar.activation(gact[:, 12:16], xprojT[:, 12:16, 0],
                                         AF.Sigmoid)
                else:
                    tiles = [g_ps.tile([128, hi - lo], F32, tag=f"ps{gi}",
                                       name=f"ps{gi}_{t}",
                                       bufs=(2 if gi == 0 else 1))
                             for gi, (lo, hi) in enumerate(groups)]
                    # x-projection preload (PE, runs during previous tail)
                    for ps, (lo, hi) in zip(tiles, groups):
                        nc.tensor.matmul(ps[:], idn[:],
                                         xprojT[:, lo:hi, t],
                                         start=True, stop=False)
                    # W_hh @ h matmuls, group-major so (i,g) closes first
                    for ps, (lo, hi) in zip(tiles, groups):
                        for j in range(lo, hi):
                            for k in range(4):
                                nc.tensor.matmul(
                                    ps[:, j - lo:j - lo + 1],
                                    whhT[:, k * 16 + j, :],
                                    hhist[:, k, t - 1:t],
                                    start=False,
                                    stop=(j == hi - 1 and k == 3))
                    for ps, (lo, hi) in zip(tiles, groups):
                        nc.scalar.activation(gact[:, lo:hi], ps[:],
                                             AF.Sigmoid)
                # g' = 2*sigmoid(2a_g) - 1 = tanh(a_g)
                gp = spool.tile([128, 4], F32, tag="gp")
                nc.vector.tensor_scalar(gp[:], gact[:, 4:8], 2.0, -1.0,
                                        ALU.mult, ALU.add)
                ig = spool.tile([128, 4], F32, tag="ig")
                nc.vector.tensor_mul(ig[:], gact[:, 0:4], gp[:])
                fc = spool.tile([128, 4], F32, tag="fc")
                nc.vector.tensor_mul(fc[:], gact[:, 8:12], c_sb[:])
                nc.vector.tensor_add(c_sb[:], ig[:], fc[:])
                tc_t = spool.tile([128, 4], F32, tag="tc")
                nc.scalar.activation(tc_t[:], c_sb[:], AF.Tanh)
                nc.vector.tensor_mul(hhist[:, :, t], gact[:, 12:16], tc_t[:])

            # ---- phase 3: per-core step-block softmax head ----
            cid = nc.vector.partition_id()
            off = cid * sblk
            nc.vector.tensor_copy(hblk[:], hhist[:, :, bass.ds(off, sblk)])
            for n in range(NB):
                if n in wts:
                    wt = wts.pop(n)
                else:
                    wt = wpool.tile([128, 4, NW], BF16, name=f"wt_{n}",
                                    tag="wt")
                    eng = nc.sync if n % 2 == 0 else nc.gpsimd
                    eng.dma_start(wt[:], woutT_r[:, :, n * NW:(n + 1) * NW])
                bt = bpool.tile([1, NW], BF16)
                nc.gpsimd.dma_start(bt[:], bout_d[0:1, n * NW:(n + 1) * NW])
                ps = lg_ps.tile([128, NW], F32)
                nc.tensor.matmul(ps[:sblk, :], ones1[0:1, 0:sblk], bt[:],
                                 start=True, stop=False)
                for k in range(4):
                    nc.tensor.matmul(ps[:sblk, :], hblk[:, k, :], wt[:, k, :],
                                     start=False, stop=(k == 3))
                nc.scalar.activation(exps[:sblk, n * NW:(n + 1) * NW],
                                     ps[:sblk, :], AF.Exp,
                                     accum_out=sums[:sblk, n:n + 1])
            nc.vector.reduce_sum(tot[:sblk, :], sums[:sblk, :],
                                 axis=mybir.AxisListType.X)
            nc.vector.reciprocal(inv[:sblk, :], tot[:sblk, :])
            # normalize + write out in 4-block chunks: fewer, larger DMAs
            OW = 4 * NW
            for n4 in range(NB // 4):
                ot = opool.tile([128, OW], F32)
                nc.vector.tensor_scalar_mul(
                    ot[:sblk, :],
                    exps[:sblk, n4 * OW:(n4 + 1) * OW],
                    inv[:sblk, :])
                eng = nc.sync if n4 % 2 == 0 else nc.gpsimd
                eng.dma_start(probs_d.ap()[:, n4 * OW:(n4 + 1) * OW],
                              ot[:sblk, :])
    nc.compile()
    return nc


def prep_inputs(features, captions, emb, W_ih, W_hh, b_ih, b_hh, W_out, b_out,
                steps=S):
    """Host-side packing: gather + transpose + gate permutation. Pure data
    movement (plus the 2x fold for the tanh-via-sigmoid identity); all FLOPs
    stay on device."""
    features = np.asarray(features, np.float32)
    captions = np.asarray(captions)
    emb = np.asarray(emb, np.float32)
    W_ih = np.asarray(W_ih, np.float32)
    W_hh = np.asarray(W_hh, np.float32)
    W_out = np.asarray(W_out, np.float32)
    b = np.asarray(b_ih, np.float32) + np.asarray(b_hh, np.float32)
    b_out = np.asarray(b_out, np.float32)

    # gate order [i,f,g,o] -> [i,g,f,o]; double the g rows so that
    # tanh(a_g) = 2*sigmoid(2*a_g) - 1 needs only a sigmoid on device
    perm = np.concatenate([np.arange(0, 512), np.arange(1024, 1536),
                           np.arange(512, 1024), np.arange(1536, 2048)])
    scale = np.ones((2048, 1), np.float32)
    scale[512:1024] = 2.0
    Wih_p = W_ih[perm] * scale
    Whh_p = W_hh[perm] * scale
    b_p = b[perm] * scale[:, 0]

    xs = np.concatenate([features[:, None, :], emb[captions]], axis=1)
    xs = xs.reshape(S, E)[:steps]
    xsT = np.ascontiguousarray(
        xs.T.reshape(2, 128, steps).transpose(1, 0, 2)).astype(BF)  # [p,e,t]
    wihT = np.ascontiguousarray(
        Wih_p.T.reshape(2, 128, 16, 128).transpose(1, 0, 2, 3)
        .reshape(128, 32, 128)).astype(BF)                        # [p,(e,j),m]
    biasg = np.ascontiguousarray(b_p.reshape(16, 128).T)          # [p,j]
    whhT = np.ascontiguousarray(
        Whh_p.T.reshape(4, 128, 16, 128).transpose(1, 0, 2, 3)
        .reshape(128, 64, 128)).astype(BF)                        # [p,(k,j),m]
    woutT = np.ascontiguousarray(W_out.T.reshape(4, 128, V)).astype(BF)
    bout = b_out[None, :].astype(BF)
    ones1 = np.ones((1, 128), BF)
    idn = np.eye(128, dtype=np.float32).astype(BF)
    return {"xsT": xsT, "wihT": wihT, "biasg": biasg, "whhT": whhT,
            "woutT": woutT, "bout": bout, "ones1": ones1, "idn": idn}


_NC_CACHE = {}


def _get_nc(steps=S):
    if steps not in _NC_CACHE:
        _NC_CACHE[steps] = build_nc(steps)
    return _NC_CACHE[steps]


def kernel(**inputs):
    from concourse.bass_utils import run_bass_kernel_spmd
    nc = _get_nc(S)
    in_map = prep_inputs(**inputs)
    res = run_bass_kernel_spmd(nc, [dict(in_map) for _ in range(N_CORES)],
                               core_ids=list(range(N_CORES)))
    probs = np.concatenate([res.results[c]["probs"] for c in range(N_CORES)],
                           axis=0)
    return probs.reshape(B, T + 1, V).astype(np.float32)

